# revision 33
# baseline (speedup 1.0000x reference)
"""MultiHeadSelectiveAttention TRN2 kernel: FULL inputs -> FULL output.

Shards batch (B=8) across 8 NeuronCores (data-parallel, one batch element
per core). Per batch b, using the value-head-dim-1 collapse:
    v   = x Wv + bv                                     [L, H]
    xv  = x^T v                                         [D, H]
    ktv = blockdiag_mask(Wk^T xv + bk (x) sum_l v)      [D, H]
    u   = Wq ktv ;  c[h] = bq . ktv[:, h]
    out = sigmoid((x u + c)/8)^T * mask                 [H, L]
identical in exact arithmetic to the reference attention.

v2: all M=16 matmuls are 4-way column-tiled (tile_position col strips at
partition bases 0/32/64/96 stream concurrently through the PE), x ships
only in transposed layout (natural-layout tiles are produced by on-chip
PE transposes, which pipeline at ~57ns), and Wk/Wq^T ship d-major in
quarters so step3/step4/z overlap the tail of the weight stream.
All matmul operands fp16; accumulation fp32 in PSUM.
"""
import sys
sys.path.insert(0, '/opt/trn_rl_repo')
sys.path.insert(0, '/root/problem')
from contextlib import ExitStack
import numpy as np
import concourse.bass as bass
import concourse.tile as tile
import concourse.mybir as mybir
from concourse.masks import make_identity
import waitfix  # noqa: F401  (multi-wait codegen workaround)

f32 = mybir.dt.float32
f16 = mybir.dt.float16
Sigmoid = mybir.ActivationFunctionType.Sigmoid
Copy = mybir.ActivationFunctionType.Copy

L, D, H = 4096, 1024, 16
NDT = D // 128        # 8 contraction chunks
NQ = 8                # L eighths (512 rows each)
NLT = 32              # L tiles (128 rows each)


def build():
    nc = bass.Bass(trn_type="TRN2")
    # x^T: col = 4096*q + 512*c + lq   (eighth q, chunk c, lq in [0,512))
    xt_d = nc.dram_tensor("xt", [128, NQ * 4096], f16, kind="ExternalInput")
    # wk/wqt d-major quarters: col = 2048*jw + 256*c + n'
    wk_d = nc.dram_tensor("wk", [128, 8192], f16, kind="ExternalInput")
    wqt_d = nc.dram_tensor("wqt", [128, 8192], f16, kind="ExternalInput")
    wv_d = nc.dram_tensor("wvr", [128, NDT * H], f16, kind="ExternalInput")
    bvr_d = nc.dram_tensor("bvr", [128, H], f32, kind="ExternalInput")
    bqr_d = nc.dram_tensor("bqr", [128, NDT], f16, kind="ExternalInput")
    bvc_d = nc.dram_tensor("bvc", [H, 1], f32, kind="ExternalInput")
    bkr_d = nc.dram_tensor("bkr", [128, 256], f32, kind="ExternalInput")
    bdm_d = nc.dram_tensor("bdm", [128, NDT * H], f32, kind="ExternalInput")
    out0_d = nc.dram_tensor("out0", [128, 512], f16, kind="ExternalOutput")
    out1_d = nc.dram_tensor("out1", [128, 512], f16, kind="ExternalOutput")

    with ExitStack() as ctx:
        tc = ctx.enter_context(tile.TileContext(nc))
        konst = ctx.enter_context(tc.tile_pool(name="konst", bufs=1))
        pers = ctx.enter_context(tc.tile_pool(name="pers", bufs=1))
        xtp = ctx.enter_context(tc.tile_pool(name="xtp", bufs=1))
        wgt = ctx.enter_context(tc.tile_pool(name="wgt", bufs=1))

        # ---- the big ordered sync-ring stream, emitted FIRST so the sync
        # engine's queue leads with it: x^T chunks, then wk/wqt quarters.
        xtall = xtp.tile([128, NQ * 4096], f16)
        for q in range(NQ):
            nc.sync.dma_start(xtall[:, 4096 * q:4096 * (q + 1)],
                              xt_d[:, 4096 * q:4096 * (q + 1)])
        wk_sb = wgt.tile([128, 8192], f16)
        wqt_sb = wgt.tile([128, 8192], f16)
        for hh in range(2):
            nc.sync.dma_start(wk_sb[:, 4096 * hh:4096 * (hh + 1)],
                              wk_d[:, 4096 * hh:4096 * (hh + 1)])
        for jw in range(4):
            nc.sync.dma_start(wqt_sb[:, 2048 * jw:2048 * (jw + 1)],
                              wqt_d[:, 2048 * jw:2048 * (jw + 1)])

        # PE warm-up: HAM to 8/8 before the first real matmul, using
        # a DVE-memset dummy (no slow gpsimd const prep on this path).
        dummy = konst.tile([128, 512], f16)
        nc.vector.memset(dummy[:], 0.0)
        with tc.tile_pool(name="ps_wu", bufs=1, space="PSUM") as ps_wu:
            psw = ps_wu.tile([128, 256], f32, tag="wu")
            for _ in range(16):
                nc.tensor.matmul(psw[:], dummy[:, 0:128], dummy[:, 0:256],
                                 start=True, stop=True, skip_group_check=True)

        # ---------------- constants (scalar ring) ----------------
        ident = konst.tile([128, 128], f32)
        make_identity(nc, ident[:])
        ident16 = konst.tile([128, 128], f16)
        nc.vector.tensor_copy(ident16[:], ident[:])
        wvr = konst.tile([128, NDT * H], f16)
        nc.scalar.dma_start(wvr[:], wv_d[:, :])
        bvr = konst.tile([128, H], f32)
        nc.scalar.dma_start(bvr[:], bvr_d[:, :])
        bqr = konst.tile([128, NDT], f16)
        nc.scalar.dma_start(bqr[:], bqr_d[:, :])
        bvc = konst.tile([H, 1], f32)
        nc.scalar.dma_start(bvc[:], bvc_d[:, :])
        bkr = konst.tile([128, 256], f32)
        nc.scalar.dma_start(bkr[:], bkr_d[:, :])
        bdm = konst.tile([128, NDT * H], f32)
        nc.scalar.dma_start(bdm[:], bdm_d[:, :])

        def idq(j):
            """identity [16,16] block at base partition 32j."""
            return ident16[32 * j:32 * j + 16, 32 * j:32 * j + 16]

        def xts(c, q, off=0, w=512):
            """x^T slice [128, w] for chunk c, eighth q, offset off."""
            base = 4096 * q + 512 * c + off
            return xtall[:, base:base + w]

        svps = []
        vns = []

        # ---------------- PHASE A: v, transposes, xv ----------------
        with tc.tile_pool(name="vtsp", bufs=2) as vtsp, \
             tc.tile_pool(name="xnp", bufs=1) as xnp, \
             tc.tile_pool(name="vnp", bufs=1) as vnp, \
             tc.tile_pool(name="ps_v", bufs=1, space="PSUM") as ps_v, \
             tc.tile_pool(name="ps_t", bufs=4, space="PSUM") as ps_t, \
             tc.tile_pool(name="ps_f", bufs=1, space="PSUM") as ps_f, \
             tc.tile_pool(name="ps_xv", bufs=1, space="PSUM") as ps_xv:

            psxv = ps_xv.tile([128, 256], f32, tag="xv")
            vts = {}
            xns = {}

            def folds(q):
                # v^T strip [16,128]@base32j -> vn [128,16] via row-tiled
                # NORMAL matmul (out = strip.T @ I16); pairs of row tiles
                # into 2 psum banks, DVE bias-add trailing.
                for j in range(4):
                    psf = ps_f.tile([128, H], f32, name=f"psf{j}",
                                    tag=f"psf{j % 2}")
                    nc.tensor.matmul(
                        psf[:], vts[q][32 * j:32 * j + 16, 0:128], idq(j),
                        start=True, stop=True,
                        skip_group_check=True, tile_position=(32 * j, 0))
                    vn = vnp.tile([128, H], f16, name=f"vn{4 * q + j}",
                                  tag=f"vn{4 * q + j}", bufs=1)
                    nc.vector.tensor_add(vn[:], psf[:], bvr[:])
                    vns.append(vn)

            def xvmms(q):
                for j in range(4):
                    lt = 4 * q + j
                    for jd in range(4):
                        nc.tensor.matmul(
                            psxv[32 * jd:32 * jd + 16, :],
                            vns[lt][:], xns[lt][:, 256 * jd:256 * jd + 256],
                            start=(lt == 0), stop=(lt == NLT - 1),
                            tile_position=(0, 32 * jd),
                            skip_group_check=True)

            for q in range(NQ):
                # v^T for eighth q: 4 concurrent 128-L strips
                psv = ps_v.tile([128, 128], f32, tag="v")
                for c in range(NDT):
                    for j in range(4):
                        nc.tensor.matmul(
                            psv[32 * j:32 * j + 16, :],
                            wvr[:, H * c:H * c + H],
                            xts(c, q, 128 * j, 128),
                            start=(c == 0), stop=(c == NDT - 1),
                            tile_position=(0, 32 * j),
                            skip_group_check=True)
                vt = vtsp.tile([128, 128], f16, tag="vts")
                svp = pers.tile([128, 1], f32, name=f"svp{q}", tag=f"svp{q}")
                nc.scalar.activation(vt[:], psv[:], Copy, accum_out=svp[:])
                vts[q] = vt
                svps.append(svp)

                if q >= 1:
                    folds(q - 1)

                # x^T -> x natural transposes for eighth q's 4 L-tiles
                for j in range(4):
                    lt = 4 * q + j
                    pst = ps_t.tile([128, 1024], f16, tag="t")
                    for c in range(NDT):
                        nc.tensor.matmul(
                            pst[:, 128 * c:128 * c + 128],
                            xts(c, q, 128 * j, 128), ident16[:],
                            start=True, stop=True, is_transpose=True,
                            skip_group_check=True)
                    xn = xnp.tile([128, 1024], f16, tag=f"xn{lt % 8}")
                    nc.vector.tensor_copy(xn[:, 0:704], pst[:, 0:704])
                    nc.scalar.copy(xn[:, 704:1024], pst[:, 704:1024])
                    xns[lt] = xn

                if q >= 1:
                    xvmms(q - 1)
            folds(NQ - 1)
            psa = ps_v.tile([128, 128], f32, name="psa", tag="v")
            for _ in range(10):
                nc.tensor.matmul(psa[:, 0:16], dummy[:, 0:128],
                                 dummy[:, 0:16], start=True, stop=True,
                                 skip_group_check=True)
            xvmms(NQ - 1)

            # -------- A -> B transition (still inside A pools) --------
            xvt = pers.tile([128, 256], f16, tag="xvt")
            nc.vector.tensor_copy(xvt[:], psxv[:])

        # sv = sum_l v = strip/eighth partials + L*bv
        svt = pers.tile([128, 1], f32, tag="svt")
        nc.vector.tensor_add(svt[:], svps[0][:], svps[1][:])
        for q in range(2, NQ):
            nc.vector.tensor_add(svt[:], svt[:], svps[q][:])
        sv16 = pers.tile([H, 1], f32, tag="sv16")
        svf = []
        for j in range(1, 4):
            t = pers.tile([H, 1], f32, name=f"svf{j}", tag=f"svf{j}")
            nc.vector.tensor_copy(t[:], svt[32 * j:32 * j + 16, :])
            svf.append(t)
        nc.vector.tensor_add(sv16[:], svt[0:16, :], svf[0][:])
        nc.vector.tensor_add(sv16[:], sv16[:], svf[1][:])
        nc.vector.tensor_add(sv16[:], sv16[:], svf[2][:])
        bvl = pers.tile([H, 1], f32, tag="bvl")
        nc.scalar.mul(bvl[:], bvc[:], float(L))
        nc.vector.tensor_add(sv16[:], sv16[:], bvl[:])
        svst = pers.tile([128, 1], f32, tag="svst")
        for j in range(4):
            nc.vector.tensor_copy(svst[32 * j:32 * j + 16, :], sv16[:])
        bksv = pers.tile([128, 256], f32, tag="bksv")
        nc.scalar.activation(bksv[:], bkr[:], Copy, scale=svst[:])

        # ---------------- PHASE B ----------------
        with tc.tile_pool(name="sbB", bufs=1) as sbB, \
             tc.tile_pool(name="ps_3", bufs=1, space="PSUM") as ps_3, \
             tc.tile_pool(name="ps_4", bufs=1, space="PSUM") as ps_4, \
             tc.tile_pool(name="ps_m", bufs=2, space="PSUM") as ps_m, \
             tc.tile_pool(name="ps_c", bufs=1, space="PSUM") as ps_c, \
             tc.tile_pool(name="ps_z", bufs=1, space="PSUM") as ps_z:

            ktvt = sbB.tile([128, 256], f16, tag="ktvt")
            ut = sbB.tile([128, 256], f16, tag="ut")
            xvn = []
            ktvn = []
            un = []

            def warm(n):
                """dummy MMs to bridge PE waits on DVE/ACT evacs (keeps the
                stream dense and HAM at 8/8)."""
                psm = ps_m.tile([128, H], f32, tag="m")
                for _ in range(n):
                    nc.tensor.matmul(psm[:], dummy[:, 0:128], dummy[:, 0:16],
                                     start=True, stop=True,
                                     skip_group_check=True)

            warm(8)

            def tp16(src, d, out_name):
                """[16,128] strip-slice (chunk d) -> [128,16] via row-tiled
                normal matmul (out = slice.T @ I16)."""
                j = d // 2
                e = d % 2
                psm = ps_m.tile([128, H], f32, tag="m")
                nc.tensor.matmul(
                    psm[:], src[32 * j:32 * j + 16, 128 * e:128 * e + 128],
                    idq(j), start=True, stop=True,
                    skip_group_check=True, tile_position=(32 * j, 0))
                t = sbB.tile([128, H], f16, name=out_name, tag=out_name,
                             bufs=1)
                return psm, t

            # xv natural chunks (warm-fill between the strip folds)
            for d in range(NDT):
                psm, t = tp16(xvt, d, f"xvn{d}")
                nc.vector.tensor_copy(t[:], psm[:])
                xvn.append(t)
                warm(2)

            # step3 dense col-tiled (4 strips concurrent), one evac
            ps3 = ps_3.tile([128, 256], f32, name="ps3", tag="s3")
            for c in range(NDT):
                for jw in range(4):
                    nc.tensor.matmul(
                        ps3[32 * jw:32 * jw + 16, :], xvn[c][:],
                        wk_sb[:, 2048 * jw + 256 * c:2048 * jw + 256 * c + 256],
                        start=(c == 0), stop=(c == NDT - 1),
                        tile_position=(0, 32 * jw), skip_group_check=True)
            nc.vector.tensor_add(ktvt[:], ps3[:], bksv[:])
            warm(10)

            # masked ktv natural chunks
            for d in range(NDT):
                psm, t = tp16(ktvt, d, f"ktvn{d}")
                nc.vector.tensor_mul(t[:], psm[:], bdm[:, H * d:H * d + H])
                ktvn.append(t)
                warm(2)

            # c = (bq . ktv_m)/8 -> strip-layout bias
            psc = ps_c.tile([H, 1], f32, tag="c")
            for c in range(NDT):
                nc.tensor.matmul(psc[:], ktvn[c][:], bqr[:, c:c + 1],
                                 start=(c == 0), stop=(c == NDT - 1))
            c16 = sbB.tile([H, 1], f32, tag="c16")
            nc.scalar.copy(c16[:], psc[:])
            nc.scalar.mul(c16[:], c16[:], 0.125)
            cst = sbB.tile([128, 1], f32, tag="cst")
            for j in range(4):
                nc.vector.tensor_copy(cst[32 * j:32 * j + 16, :], c16[:])

            # step4 quarter waves (strip jw gated on wqt quarter jw); u
            # folds and z chunk-pair MMs trail each wave so z hides under
            # the tail of the weight stream; warm-fill keeps HAM at 8/8.
            psz0 = ps_z.tile([128, 512], f32, tag="z0")
            psz1 = ps_z.tile([128, 512], f32, tag="z1")

            def zp(cc, ps, half):
                for j in range(4):
                    nc.tensor.matmul(
                        ps[32 * j:32 * j + 16, :], un[cc][:],
                        xts(cc, 2 * j + half), start=(cc == 0),
                        stop=(cc == NDT - 1),
                        tile_position=(0, 32 * j), skip_group_check=True)

            ps4 = ps_4.tile([128, 256], f32, name="ps4", tag="s4")
            sg0 = sbB.tile([128, 512], f16, tag="sg0")
            sg1 = sbB.tile([128, 512], f16, tag="sg1")
            for jw in range(4):
                for c in range(NDT):
                    nc.tensor.matmul(
                        ps4[32 * jw:32 * jw + 16, :], ktvn[c][:],
                        wqt_sb[:, 2048 * jw + 256 * c:2048 * jw + 256 * c + 256],
                        start=(c == 0), stop=(c == NDT - 1),
                        tile_position=(0, 32 * jw), skip_group_check=True)
                nc.vector.tensor_copy(ut[32 * jw:32 * jw + 16, :],
                                      ps4[32 * jw:32 * jw + 16, :])
                warm(3)
                for d in (2 * jw, 2 * jw + 1):
                    psm, t = tp16(ut, d, f"un{d}")
                    nc.vector.tensor_copy(t[:], psm[:])
                    un.append(t)
                    warm(2)
                for cc in (2 * jw, 2 * jw + 1):
                    if cc < NDT - 1:
                        zp(cc, psz0, 0)
                        zp(cc, psz1, 1)
            # final chunk: finish psz0, sigmoid+store half 0 while psz1's
            # last group and half-1 sigmoid+store trail
            zp(NDT - 1, psz0, 0)
            nc.scalar.activation(sg0[:], psz0[:], Sigmoid, bias=cst[:],
                                 scale=0.125)
            nc.sync.dma_start(out0_d[:, :], sg0[:])
            zp(NDT - 1, psz1, 1)
            nc.scalar.activation(sg1[:], psz1[:], Sigmoid, bias=cst[:],
                                 scale=0.125)
            nc.scalar.dma_start(out1_d[:, :], sg1[:])
    return nc


B = 8
_cache = {}


def _get_nc():
    if "nc" not in _cache:
        _cache["nc"] = build()
    return _cache["nc"]


def build_in_maps(x, mask, Wq, bq, Wk, bk, Wv, bv):
    x16 = np.asarray(x).astype(np.float16)
    Wq = np.asarray(Wq, dtype=np.float32)
    Wk = np.asarray(Wk, dtype=np.float32)
    Wv = np.asarray(Wv, dtype=np.float32)
    bq = np.asarray(bq, dtype=np.float32)
    bk = np.asarray(bk, dtype=np.float32)
    bv = np.asarray(bv, dtype=np.float32)
    # wk/wqt d-major quarters: [p, (jw c n')]; row 128c+p, col 256jw+n'
    wk16 = np.ascontiguousarray(
        Wk.astype(np.float16).reshape(NDT, 128, 4, 256)
        .transpose(1, 2, 0, 3).reshape(128, 8192))
    wqt16 = np.ascontiguousarray(
        Wq.T.astype(np.float16).reshape(NDT, 128, 4, 256)
        .transpose(1, 2, 0, 3).reshape(128, 8192))
    wvr = np.ascontiguousarray(
        Wv.reshape(NDT, 128, H).transpose(1, 0, 2).reshape(128, NDT * H)
    ).astype(np.float16)
    bvr = np.ascontiguousarray(
        np.broadcast_to(bv[None, :], (128, H))).astype(np.float32)
    bqr = np.ascontiguousarray(bq.reshape(NDT, 128).T).astype(np.float16)
    bvc = np.ascontiguousarray(bv.reshape(H, 1))
    # bk strip layout [32j+h, n'] = bk[256j+n']
    bkr = np.ascontiguousarray(
        np.broadcast_to(bk.reshape(4, 1, 256), (4, 32, 256))
        .reshape(128, 256).astype(np.float32))
    # blockdiag masks, ktv-natural chunk-major [p, 16d+h]
    bdm = np.zeros((128, NDT * H), dtype=np.float32)
    for d in range(NDT):
        bdm[0:64, H * d + 2 * d] = 1.0
        bdm[64:128, H * d + 2 * d + 1] = 1.0
    in_maps = []
    for b in range(B):
        # xt: [p, (q c lq)] — x^T row 128c+p, col 512q+lq
        xtr = np.ascontiguousarray(
            x16[b].T.reshape(NDT, 128, NQ, 512)
            .transpose(1, 2, 0, 3).reshape(128, NQ * NDT * 512))
        in_maps.append({
            "xt": xtr, "wk": wk16, "wqt": wqt16, "wvr": wvr,
            "bvr": bvr, "bqr": bqr, "bvc": bvc, "bkr": bkr, "bdm": bdm,
        })
    return in_maps


def kernel(x, mask, Wq, bq, Wk, bk, Wv, bv):
    from concourse.bass_utils import run_bass_kernel_spmd
    nc = _get_nc()
    in_maps = build_in_maps(x, mask, Wq, bq, Wk, bk, Wv, bv)
    res = run_bass_kernel_spmd(nc, in_maps, core_ids=list(range(B)))
    outs = []
    for b in range(B):
        sg0 = np.asarray(res.results[b]["out0"], dtype=np.float32)
        sg1 = np.asarray(res.results[b]["out1"], dtype=np.float32)
        sg = np.concatenate([sg0, sg1], axis=1)      # [128, 1024]
        # row 32j+h, col n -> z[h, 1024j + n]
        zb = sg.reshape(4, 32, 1024)[:, 0:16].transpose(1, 0, 2).reshape(H, L)
        outs.append(zb)
    out = np.stack(outs, axis=0)
    out = out * np.asarray(mask).astype(np.float32)[:, None, :]
    return out.astype(np.float32)


# revision 34
# speedup vs baseline: 1.1112x; 1.1112x over previous
"""MultiHeadSelectiveAttention TRN2 kernel: FULL inputs -> FULL output.

Shards batch (B=8) across 8 NeuronCores (data-parallel, one batch element
per core). Per batch b, using the value-head-dim-1 collapse:
    v   = x Wv + bv                                     [L, H]
    xv  = x^T v                                         [D, H]
    ktv = blockdiag_mask(Wk^T xv + bk (x) sum_l v)      [D, H]
    u   = Wq ktv ;  c[h] = bq . ktv[:, h]
    out = sigmoid((x u + c)/8)^T * mask                 [H, L]
identical in exact arithmetic to the reference attention.

v2: all M=16 matmuls are 4-way column-tiled (tile_position col strips at
partition bases 0/32/64/96 stream concurrently through the PE), x ships
only in transposed layout (natural-layout tiles are produced by on-chip
PE transposes, which pipeline at ~57ns), and Wk/Wq^T ship d-major in
quarters so step3/step4/z overlap the tail of the weight stream.
All matmul operands fp16; accumulation fp32 in PSUM.
"""
import sys
sys.path.insert(0, '/opt/trn_rl_repo')
sys.path.insert(0, '/root/problem')
from contextlib import ExitStack
import numpy as np
import concourse.bass as bass
import concourse.tile as tile
import concourse.mybir as mybir
from concourse.masks import make_identity
import waitfix  # noqa: F401  (multi-wait codegen workaround)

f32 = mybir.dt.float32
f16 = mybir.dt.float16
Sigmoid = mybir.ActivationFunctionType.Sigmoid
Copy = mybir.ActivationFunctionType.Copy

L, D, H = 4096, 1024, 16
NDT = D // 128        # 8 contraction chunks
NQ = 8                # L eighths (512 rows each)
NLT = 32              # L tiles (128 rows each)


def build():
    nc = bass.Bass(trn_type="TRN2")
    # x^T: col = 4096*q + 512*c + lq   (eighth q, chunk c, lq in [0,512))
    xt_d = nc.dram_tensor("xt", [128, NQ * 4096], f16, kind="ExternalInput")
    # wk/wqt d-major quarters: col = 2048*jw + 256*c + n'
    wk_d = nc.dram_tensor("wk", [128, 8192], f16, kind="ExternalInput")
    wqt_d = nc.dram_tensor("wqt", [128, 8192], f16, kind="ExternalInput")
    wv_d = nc.dram_tensor("wvr", [128, NDT * H], f16, kind="ExternalInput")
    bvr_d = nc.dram_tensor("bvr", [128, H], f32, kind="ExternalInput")
    bqr_d = nc.dram_tensor("bqr", [128, NDT], f16, kind="ExternalInput")
    bvc_d = nc.dram_tensor("bvc", [H, 1], f32, kind="ExternalInput")
    bkr_d = nc.dram_tensor("bkr", [128, 256], f32, kind="ExternalInput")
    bdm_d = nc.dram_tensor("bdm", [128, NDT * H], f32, kind="ExternalInput")
    out0_d = nc.dram_tensor("out0", [128, 512], f16, kind="ExternalOutput")
    out1_d = nc.dram_tensor("out1", [128, 512], f16, kind="ExternalOutput")

    with ExitStack() as ctx:
        tc = ctx.enter_context(tile.TileContext(nc))
        konst = ctx.enter_context(tc.tile_pool(name="konst", bufs=1))
        pers = ctx.enter_context(tc.tile_pool(name="pers", bufs=1))
        xtp = ctx.enter_context(tc.tile_pool(name="xtp", bufs=1))
        wgt = ctx.enter_context(tc.tile_pool(name="wgt", bufs=1))

        # ---- the big ordered sync-ring stream, emitted FIRST so the sync
        # engine's queue leads with it: x^T chunks, then wk/wqt quarters.
        xtall = xtp.tile([128, NQ * 4096], f16)
        for q in range(NQ):
            nc.sync.dma_start(xtall[:, 4096 * q:4096 * (q + 1)],
                              xt_d[:, 4096 * q:4096 * (q + 1)])
        wk_sb = wgt.tile([128, 8192], f16)
        wqt_sb = wgt.tile([128, 8192], f16)
        for hh in range(2):
            nc.sync.dma_start(wk_sb[:, 4096 * hh:4096 * (hh + 1)],
                              wk_d[:, 4096 * hh:4096 * (hh + 1)])
        for jw in range(4):
            nc.sync.dma_start(wqt_sb[:, 2048 * jw:2048 * (jw + 1)],
                              wqt_d[:, 2048 * jw:2048 * (jw + 1)])

        # PE warm-up: HAM to 8/8 before the first real matmul, using
        # a DVE-memset dummy (no slow gpsimd const prep on this path).
        dummy = konst.tile([128, 512], f16)
        nc.vector.memset(dummy[:], 0.0)
        with tc.tile_pool(name="ps_wu", bufs=1, space="PSUM") as ps_wu:
            psw = ps_wu.tile([128, 256], f32, tag="wu")
            for _ in range(16):
                nc.tensor.matmul(psw[:], dummy[:, 0:128], dummy[:, 0:256],
                                 start=True, stop=True, skip_group_check=True)

        # ---------------- constants (scalar ring) ----------------
        ident = konst.tile([128, 128], f32)
        make_identity(nc, ident[:])
        ident16 = konst.tile([128, 128], f16)
        nc.vector.tensor_copy(ident16[:], ident[:])
        wvr = konst.tile([128, NDT * H], f16)
        nc.scalar.dma_start(wvr[:], wv_d[:, :])
        bvr = konst.tile([128, H], f32)
        nc.scalar.dma_start(bvr[:], bvr_d[:, :])
        bqr = konst.tile([128, NDT], f16)
        nc.scalar.dma_start(bqr[:], bqr_d[:, :])
        bvc = konst.tile([H, 1], f32)
        nc.scalar.dma_start(bvc[:], bvc_d[:, :])
        bkr = konst.tile([128, 256], f32)
        nc.scalar.dma_start(bkr[:], bkr_d[:, :])
        bdm = konst.tile([128, NDT * H], f32)
        nc.scalar.dma_start(bdm[:], bdm_d[:, :])

        def idq(j):
            """identity [16,16] block at base partition 32j."""
            return ident16[32 * j:32 * j + 16, 32 * j:32 * j + 16]

        def xts(c, q, off=0, w=512):
            """x^T slice [128, w] for chunk c, eighth q, offset off."""
            base = 4096 * q + 512 * c + off
            return xtall[:, base:base + w]

        svps = []
        vns = []

        # ---------------- PHASE A: v, transposes, xv ----------------
        with tc.tile_pool(name="vtsp", bufs=2) as vtsp, \
             tc.tile_pool(name="xnp", bufs=1) as xnp, \
             tc.tile_pool(name="vnp", bufs=1) as vnp, \
             tc.tile_pool(name="ps_v", bufs=1, space="PSUM") as ps_v, \
             tc.tile_pool(name="ps_t", bufs=4, space="PSUM") as ps_t, \
             tc.tile_pool(name="ps_f", bufs=1, space="PSUM") as ps_f, \
             tc.tile_pool(name="ps_xv", bufs=1, space="PSUM") as ps_xv:

            psxv = ps_xv.tile([128, 256], f32, tag="xv")
            vts = {}
            xns = {}

            def folds(q):
                # v^T strip [16,128]@base32j -> vn [128,16] via row-tiled
                # NORMAL matmul (out = strip.T @ I16); pairs of row tiles
                # into 2 psum banks, DVE bias-add trailing.
                for j in range(4):
                    psf = ps_f.tile([128, H], f32, name=f"psf{j}",
                                    tag=f"psf{j % 2}")
                    nc.tensor.matmul(
                        psf[:], vts[q][32 * j:32 * j + 16, 0:128], idq(j),
                        start=True, stop=True,
                        skip_group_check=True, tile_position=(32 * j, 0))
                    vn = vnp.tile([128, H], f16, name=f"vn{4 * q + j}",
                                  tag=f"vn{4 * q + j}", bufs=1)
                    nc.vector.tensor_add(vn[:], psf[:], bvr[:])
                    vns.append(vn)

            def xvmms(q):
                for j in range(4):
                    lt = 4 * q + j
                    for jd in range(4):
                        nc.tensor.matmul(
                            psxv[32 * jd:32 * jd + 16, :],
                            vns[lt][:], xns[lt][:, 256 * jd:256 * jd + 256],
                            start=(lt == 0), stop=(lt == NLT - 1),
                            tile_position=(0, 32 * jd),
                            skip_group_check=True)

            for q in range(NQ):
                # v^T for eighth q: 4 concurrent 128-L strips
                psv = ps_v.tile([128, 128], f32, tag="v")
                for c in range(NDT):
                    for j in range(4):
                        nc.tensor.matmul(
                            psv[32 * j:32 * j + 16, :],
                            wvr[:, H * c:H * c + H],
                            xts(c, q, 128 * j, 128),
                            start=(c == 0), stop=(c == NDT - 1),
                            tile_position=(0, 32 * j),
                            skip_group_check=True)
                vt = vtsp.tile([128, 128], f16, tag="vts")
                svp = pers.tile([128, 1], f32, name=f"svp{q}", tag=f"svp{q}")
                nc.scalar.activation(vt[:], psv[:], Copy, accum_out=svp[:])
                vts[q] = vt
                svps.append(svp)

                if q >= 1:
                    folds(q - 1)

                # x^T -> x natural transposes for eighth q's 4 L-tiles
                for j in range(4):
                    lt = 4 * q + j
                    pst = ps_t.tile([128, 1024], f16, tag="t")
                    for c in range(NDT):
                        nc.tensor.matmul(
                            pst[:, 128 * c:128 * c + 128],
                            xts(c, q, 128 * j, 128), ident16[:],
                            start=True, stop=True, is_transpose=True,
                            skip_group_check=True)
                    xn = xnp.tile([128, 1024], f16, tag=f"xn{lt % 8}")
                    nc.vector.tensor_copy(xn[:, 0:704], pst[:, 0:704])
                    nc.scalar.copy(xn[:, 704:1024], pst[:, 704:1024])
                    xns[lt] = xn

                if q >= 1:
                    xvmms(q - 1)
            folds(NQ - 1)
            psa = ps_v.tile([128, 128], f32, name="psa", tag="v")
            for _ in range(10):
                nc.tensor.matmul(psa[:, 0:16], dummy[:, 0:128],
                                 dummy[:, 0:16], start=True, stop=True,
                                 skip_group_check=True)
            xvmms(NQ - 1)

            # -------- A -> B transition (still inside A pools) --------
            xvt = pers.tile([128, 256], f16, tag="xvt")
            nc.vector.tensor_copy(xvt[:], psxv[:])

        # sv = sum_l v = strip/eighth partials + L*bv
        svt = pers.tile([128, 1], f32, tag="svt")
        nc.vector.tensor_add(svt[:], svps[0][:], svps[1][:])
        for q in range(2, NQ):
            nc.vector.tensor_add(svt[:], svt[:], svps[q][:])
        sv16 = pers.tile([H, 1], f32, tag="sv16")
        svf = []
        for j in range(1, 4):
            t = pers.tile([H, 1], f32, name=f"svf{j}", tag=f"svf{j}")
            nc.vector.tensor_copy(t[:], svt[32 * j:32 * j + 16, :])
            svf.append(t)
        nc.vector.tensor_add(sv16[:], svt[0:16, :], svf[0][:])
        nc.vector.tensor_add(sv16[:], sv16[:], svf[1][:])
        nc.vector.tensor_add(sv16[:], sv16[:], svf[2][:])
        bvl = pers.tile([H, 1], f32, tag="bvl")
        nc.scalar.mul(bvl[:], bvc[:], float(L))
        nc.vector.tensor_add(sv16[:], sv16[:], bvl[:])
        svst = pers.tile([128, 1], f32, tag="svst")
        for j in range(4):
            nc.vector.tensor_copy(svst[32 * j:32 * j + 16, :], sv16[:])
        bksv = pers.tile([128, 256], f32, tag="bksv")
        nc.scalar.activation(bksv[:], bkr[:], Copy, scale=svst[:])

        # ---------------- PHASE B ----------------
        with tc.tile_pool(name="sbB", bufs=1) as sbB, \
             tc.tile_pool(name="ps_3", bufs=1, space="PSUM") as ps_3, \
             tc.tile_pool(name="ps_4", bufs=1, space="PSUM") as ps_4, \
             tc.tile_pool(name="ps_m", bufs=2, space="PSUM") as ps_m, \
             tc.tile_pool(name="ps_c", bufs=1, space="PSUM") as ps_c, \
             tc.tile_pool(name="ps_z", bufs=1, space="PSUM") as ps_z:

            ktvt = sbB.tile([128, 256], f16, tag="ktvt")
            ut = sbB.tile([128, 256], f16, tag="ut")
            xvn = []
            ktvn = []
            un = []

            def warm(n):
                """dummy MMs to bridge PE waits on DVE/ACT evacs (keeps the
                stream dense and HAM at 8/8)."""
                psm = ps_m.tile([128, H], f32, tag="m")
                for _ in range(n):
                    nc.tensor.matmul(psm[:], dummy[:, 0:128], dummy[:, 0:16],
                                     start=True, stop=True,
                                     skip_group_check=True)

            warm(8)

            def tp16(src, d, out_name):
                """[16,128] strip-slice (chunk d) -> [128,16] via row-tiled
                normal matmul (out = slice.T @ I16)."""
                j = d // 2
                e = d % 2
                psm = ps_m.tile([128, H], f32, tag="m")
                nc.tensor.matmul(
                    psm[:], src[32 * j:32 * j + 16, 128 * e:128 * e + 128],
                    idq(j), start=True, stop=True,
                    skip_group_check=True, tile_position=(32 * j, 0))
                t = sbB.tile([128, H], f16, name=out_name, tag=out_name,
                             bufs=1)
                return psm, t

            # --- B pipeline: each stage's col-tiled groups run one item
            # behind the previous stage's strip-folds, so the PE stream
            # stays dense and the DVE copies hide underneath.
            ps3 = ps_3.tile([128, 256], f32, name="ps3", tag="s3")
            ps4 = ps_4.tile([128, 256], f32, name="ps4", tag="s4")
            psz0 = ps_z.tile([128, 512], f32, tag="z0")
            psz1 = ps_z.tile([128, 512], f32, tag="z1")
            psc = ps_c.tile([H, 1], f32, tag="c")
            sg0 = sbB.tile([128, 512], f16, tag="sg0")
            sg1 = sbB.tile([128, 512], f16, tag="sg1")

            def s3g(c):
                for jw in range(4):
                    nc.tensor.matmul(
                        ps3[32 * jw:32 * jw + 16, :], xvn[c][:],
                        wk_sb[:, 2048 * jw + 256 * c:2048 * jw + 256 * c + 256],
                        start=(c == 0), stop=(c == NDT - 1),
                        tile_position=(0, 32 * jw), skip_group_check=True)

            def s4g(c):
                for jw in range(4):
                    nc.tensor.matmul(
                        ps4[32 * jw:32 * jw + 16, :], ktvn[c][:],
                        wqt_sb[:, 2048 * jw + 256 * c:2048 * jw + 256 * c + 256],
                        start=(c == 0), stop=(c == NDT - 1),
                        tile_position=(0, 32 * jw), skip_group_check=True)
                nc.tensor.matmul(psc[:], ktvn[c][:], bqr[:, c:c + 1],
                                 start=(c == 0), stop=(c == NDT - 1))

            def zp(cc, ps, half):
                for j in range(4):
                    nc.tensor.matmul(
                        ps[32 * j:32 * j + 16, :], un[cc][:],
                        xts(cc, 2 * j + half), start=(cc == 0),
                        stop=(cc == NDT - 1),
                        tile_position=(0, 32 * j), skip_group_check=True)

            # stage 1: xv folds + step3 one-behind
            for c in range(NDT):
                psm, t = tp16(xvt, c, f"xvn{c}")
                nc.vector.tensor_copy(t[:], psm[:])
                xvn.append(t)
                if c >= 1:
                    s3g(c - 1)
            s3g(NDT - 1)
            nc.vector.tensor_add(ktvt[:], ps3[:], bksv[:])
            warm(6)

            # stage 2: ktv folds + step4/c one-behind
            for d in range(NDT):
                psm, t = tp16(ktvt, d, f"ktvn{d}")
                nc.vector.tensor_mul(t[:], psm[:], bdm[:, H * d:H * d + H])
                ktvn.append(t)
                if d >= 1:
                    s4g(d - 1)
            s4g(NDT - 1)
            nc.vector.tensor_copy(ut[:], ps4[:])
            c16 = sbB.tile([H, 1], f32, tag="c16")
            nc.scalar.copy(c16[:], psc[:])
            nc.scalar.mul(c16[:], c16[:], 0.125)
            cst = sbB.tile([128, 1], f32, tag="cst")
            for j in range(4):
                nc.vector.tensor_copy(cst[32 * j:32 * j + 16, :], c16[:])
            warm(6)

            # stage 3: u folds + z chunk groups one-behind
            for d in range(NDT):
                psm, t = tp16(ut, d, f"un{d}")
                nc.vector.tensor_copy(t[:], psm[:])
                un.append(t)
                if d >= 1:
                    zp(d - 1, psz0, 0)
                    zp(d - 1, psz1, 1)
            zp(NDT - 1, psz0, 0)
            nc.scalar.activation(sg0[:], psz0[:], Sigmoid, bias=cst[:],
                                 scale=0.125)
            nc.sync.dma_start(out0_d[:, :], sg0[:])
            zp(NDT - 1, psz1, 1)
            nc.scalar.activation(sg1[:], psz1[:], Sigmoid, bias=cst[:],
                                 scale=0.125)
            nc.scalar.dma_start(out1_d[:, :], sg1[:])
    return nc


B = 8
_cache = {}


def _get_nc():
    if "nc" not in _cache:
        _cache["nc"] = build()
    return _cache["nc"]


def build_in_maps(x, mask, Wq, bq, Wk, bk, Wv, bv):
    x16 = np.asarray(x).astype(np.float16)
    Wq = np.asarray(Wq, dtype=np.float32)
    Wk = np.asarray(Wk, dtype=np.float32)
    Wv = np.asarray(Wv, dtype=np.float32)
    bq = np.asarray(bq, dtype=np.float32)
    bk = np.asarray(bk, dtype=np.float32)
    bv = np.asarray(bv, dtype=np.float32)
    # wk/wqt d-major quarters: [p, (jw c n')]; row 128c+p, col 256jw+n'
    wk16 = np.ascontiguousarray(
        Wk.astype(np.float16).reshape(NDT, 128, 4, 256)
        .transpose(1, 2, 0, 3).reshape(128, 8192))
    wqt16 = np.ascontiguousarray(
        Wq.T.astype(np.float16).reshape(NDT, 128, 4, 256)
        .transpose(1, 2, 0, 3).reshape(128, 8192))
    wvr = np.ascontiguousarray(
        Wv.reshape(NDT, 128, H).transpose(1, 0, 2).reshape(128, NDT * H)
    ).astype(np.float16)
    bvr = np.ascontiguousarray(
        np.broadcast_to(bv[None, :], (128, H))).astype(np.float32)
    bqr = np.ascontiguousarray(bq.reshape(NDT, 128).T).astype(np.float16)
    bvc = np.ascontiguousarray(bv.reshape(H, 1))
    # bk strip layout [32j+h, n'] = bk[256j+n']
    bkr = np.ascontiguousarray(
        np.broadcast_to(bk.reshape(4, 1, 256), (4, 32, 256))
        .reshape(128, 256).astype(np.float32))
    # blockdiag masks, ktv-natural chunk-major [p, 16d+h]
    bdm = np.zeros((128, NDT * H), dtype=np.float32)
    for d in range(NDT):
        bdm[0:64, H * d + 2 * d] = 1.0
        bdm[64:128, H * d + 2 * d + 1] = 1.0
    in_maps = []
    for b in range(B):
        # xt: [p, (q c lq)] — x^T row 128c+p, col 512q+lq
        xtr = np.ascontiguousarray(
            x16[b].T.reshape(NDT, 128, NQ, 512)
            .transpose(1, 2, 0, 3).reshape(128, NQ * NDT * 512))
        in_maps.append({
            "xt": xtr, "wk": wk16, "wqt": wqt16, "wvr": wvr,
            "bvr": bvr, "bqr": bqr, "bvc": bvc, "bkr": bkr, "bdm": bdm,
        })
    return in_maps


def kernel(x, mask, Wq, bq, Wk, bk, Wv, bv):
    from concourse.bass_utils import run_bass_kernel_spmd
    nc = _get_nc()
    in_maps = build_in_maps(x, mask, Wq, bq, Wk, bk, Wv, bv)
    res = run_bass_kernel_spmd(nc, in_maps, core_ids=list(range(B)))
    outs = []
    for b in range(B):
        sg0 = np.asarray(res.results[b]["out0"], dtype=np.float32)
        sg1 = np.asarray(res.results[b]["out1"], dtype=np.float32)
        sg = np.concatenate([sg0, sg1], axis=1)      # [128, 1024]
        # row 32j+h, col n -> z[h, 1024j + n]
        zb = sg.reshape(4, 32, 1024)[:, 0:16].transpose(1, 0, 2).reshape(H, L)
        outs.append(zb)
    out = np.stack(outs, axis=0)
    out = out * np.asarray(mask).astype(np.float32)[:, None, :]
    return out.astype(np.float32)


# revision 35
# speedup vs baseline: 1.1941x; 1.0746x over previous
"""MultiHeadSelectiveAttention TRN2 kernel: FULL inputs -> FULL output.

Shards batch (B=8) across 8 NeuronCores (data-parallel, one batch element
per core). Per batch b, using the value-head-dim-1 collapse:
    v   = x Wv + bv                                     [L, H]
    xv  = x^T v                                         [D, H]
    ktv = blockdiag_mask(Wk^T xv + bk (x) sum_l v)      [D, H]
    u   = Wq ktv ;  c[h] = bq . ktv[:, h]
    out = sigmoid((x u + c)/8)^T * mask                 [H, L]
identical in exact arithmetic to the reference attention.

v2: all M=16 matmuls are 4-way column-tiled (tile_position col strips at
partition bases 0/32/64/96 stream concurrently through the PE), x ships
only in transposed layout (natural-layout tiles are produced by on-chip
PE transposes, which pipeline at ~57ns), and Wk/Wq^T ship d-major in
quarters so step3/step4/z overlap the tail of the weight stream.
All matmul operands fp16; accumulation fp32 in PSUM.
"""
import sys
sys.path.insert(0, '/opt/trn_rl_repo')
sys.path.insert(0, '/root/problem')
from contextlib import ExitStack
import numpy as np
import concourse.bass as bass
import concourse.tile as tile
import concourse.mybir as mybir
from concourse.masks import make_identity
import waitfix  # noqa: F401  (multi-wait codegen workaround)

f32 = mybir.dt.float32
f16 = mybir.dt.float16
Sigmoid = mybir.ActivationFunctionType.Sigmoid
Copy = mybir.ActivationFunctionType.Copy

L, D, H = 4096, 1024, 16
NDT = D // 128        # 8 contraction chunks
NQ = 8                # L eighths (512 rows each)
NLT = 32              # L tiles (128 rows each)


def build():
    nc = bass.Bass(trn_type="TRN2")
    # x^T: col = 4096*q + 512*c + lq   (eighth q, chunk c, lq in [0,512))
    xt_d = nc.dram_tensor("xt", [128, NQ * 4096], f16, kind="ExternalInput")
    # wk/wqt d-major quarters: col = 2048*jw + 256*c + n'
    wk_d = nc.dram_tensor("wk", [128, 8192], f16, kind="ExternalInput")
    wqt_d = nc.dram_tensor("wqt", [128, 8192], f16, kind="ExternalInput")
    wv_d = nc.dram_tensor("wvr", [128, NDT * H], f16, kind="ExternalInput")
    bvr_d = nc.dram_tensor("bvr", [128, H], f32, kind="ExternalInput")
    bqr_d = nc.dram_tensor("bqr", [128, NDT], f16, kind="ExternalInput")
    bvc_d = nc.dram_tensor("bvc", [H, 1], f32, kind="ExternalInput")
    bkr_d = nc.dram_tensor("bkr", [128, 256], f32, kind="ExternalInput")
    bdm_d = nc.dram_tensor("bdm", [128, NDT * H], f32, kind="ExternalInput")
    out0_d = nc.dram_tensor("out0", [128, 512], f16, kind="ExternalOutput")
    out1_d = nc.dram_tensor("out1", [128, 512], f16, kind="ExternalOutput")

    with ExitStack() as ctx:
        tc = ctx.enter_context(tile.TileContext(nc))
        konst = ctx.enter_context(tc.tile_pool(name="konst", bufs=1))
        pers = ctx.enter_context(tc.tile_pool(name="pers", bufs=1))
        xtp = ctx.enter_context(tc.tile_pool(name="xtp", bufs=1))
        wgt = ctx.enter_context(tc.tile_pool(name="wgt", bufs=1))

        # ---- the big ordered sync-ring stream, emitted FIRST so the sync
        # engine's queue leads with it: x^T chunks, then wk/wqt quarters.
        xtall = xtp.tile([128, NQ * 4096], f16)
        for q in range(NQ):
            nc.sync.dma_start(xtall[:, 4096 * q:4096 * (q + 1)],
                              xt_d[:, 4096 * q:4096 * (q + 1)])
        wk_sb = wgt.tile([128, 8192], f16)
        wqt_sb = wgt.tile([128, 8192], f16)
        for hh in range(2):
            nc.sync.dma_start(wk_sb[:, 4096 * hh:4096 * (hh + 1)],
                              wk_d[:, 4096 * hh:4096 * (hh + 1)])
        for jw in range(4):
            nc.sync.dma_start(wqt_sb[:, 2048 * jw:2048 * (jw + 1)],
                              wqt_d[:, 2048 * jw:2048 * (jw + 1)])

        # PE warm-up: HAM to 8/8 before the first real matmul, using
        # a DVE-memset dummy (no slow gpsimd const prep on this path).
        dummy = konst.tile([128, 512], f16)
        nc.vector.memset(dummy[:], 0.0)
        with tc.tile_pool(name="ps_wu", bufs=1, space="PSUM") as ps_wu:
            psw = ps_wu.tile([128, 256], f32, tag="wu")
            for _ in range(16):
                nc.tensor.matmul(psw[:], dummy[:, 0:128], dummy[:, 0:256],
                                 start=True, stop=True, skip_group_check=True)

        # ---------------- constants (scalar ring) ----------------
        ident = konst.tile([128, 128], f32)
        make_identity(nc, ident[:])
        ident16 = konst.tile([128, 128], f16)
        nc.vector.tensor_copy(ident16[:], ident[:])
        wvr = konst.tile([128, NDT * H], f16)
        nc.scalar.dma_start(wvr[:], wv_d[:, :])
        bvr = konst.tile([128, H], f32)
        nc.scalar.dma_start(bvr[:], bvr_d[:, :])
        bqr = konst.tile([128, NDT], f16)
        nc.scalar.dma_start(bqr[:], bqr_d[:, :])
        bvc = konst.tile([H, 1], f32)
        nc.scalar.dma_start(bvc[:], bvc_d[:, :])
        bkr = konst.tile([128, 256], f32)
        nc.scalar.dma_start(bkr[:], bkr_d[:, :])
        bdm = konst.tile([128, NDT * H], f32)
        nc.scalar.dma_start(bdm[:], bdm_d[:, :])

        def idq(j):
            """identity [16,16] block at base partition 32j."""
            return ident16[32 * j:32 * j + 16, 32 * j:32 * j + 16]

        def xts(c, q, off=0, w=512):
            """x^T slice [128, w] for chunk c, eighth q, offset off."""
            base = 4096 * q + 512 * c + off
            return xtall[:, base:base + w]

        svps = []
        vns = []

        # ---------------- PHASE A: v, transposes, xv ----------------
        with tc.tile_pool(name="vtsp", bufs=2) as vtsp, \
             tc.tile_pool(name="xnp", bufs=1) as xnp, \
             tc.tile_pool(name="vnp", bufs=1) as vnp, \
             tc.tile_pool(name="ps_v", bufs=1, space="PSUM") as ps_v, \
             tc.tile_pool(name="ps_t", bufs=4, space="PSUM") as ps_t, \
             tc.tile_pool(name="ps_f", bufs=1, space="PSUM") as ps_f, \
             tc.tile_pool(name="ps_xv", bufs=1, space="PSUM") as ps_xv:

            psxv = ps_xv.tile([128, 256], f32, tag="xv")
            vts = {}
            xns = {}

            def folds(q):
                # v^T strip [16,128]@base32j -> vn [128,16] via row-tiled
                # NORMAL matmul (out = strip.T @ I16); pairs of row tiles
                # into 2 psum banks, DVE bias-add trailing.
                for j in range(4):
                    psf = ps_f.tile([128, H], f32, name=f"psf{j}",
                                    tag=f"psf{j % 2}")
                    nc.tensor.matmul(
                        psf[:], vts[q][32 * j:32 * j + 16, 0:128], idq(j),
                        start=True, stop=True,
                        skip_group_check=True, tile_position=(32 * j, 0))
                    vn = vnp.tile([128, H], f16, name=f"vn{4 * q + j}",
                                  tag=f"vn{4 * q + j}", bufs=1)
                    nc.vector.tensor_add(vn[:], psf[:], bvr[:])
                    vns.append(vn)

            def xvmms(q):
                for j in range(4):
                    lt = 4 * q + j
                    for jd in range(4):
                        nc.tensor.matmul(
                            psxv[32 * jd:32 * jd + 16, :],
                            vns[lt][:], xns[lt][:, 256 * jd:256 * jd + 256],
                            start=(lt == 0), stop=(lt == NLT - 1),
                            tile_position=(0, 32 * jd),
                            skip_group_check=True)

            def ttile(q, j):
                """x^T -> x natural transposes for L-tile 4q+j (+ evacs)."""
                lt = 4 * q + j
                pst = ps_t.tile([128, 1024], f16, tag="t")
                for c in range(NDT):
                    nc.tensor.matmul(
                        pst[:, 128 * c:128 * c + 128],
                        xts(c, q, 128 * j, 128), ident16[:],
                        start=True, stop=True, is_transpose=True,
                        skip_group_check=True)
                xn = xnp.tile([128, 1024], f16, tag=f"xn{lt % 8}")
                nc.vector.tensor_copy(xn[:, 0:704], pst[:, 0:704])
                nc.scalar.copy(xn[:, 704:1024], pst[:, 704:1024])
                xns[lt] = xn

            for q in range(NQ):
                # interleave the eighth's 4 transpose-tiles with the v
                # strips / folds / xv so the evac engines always have PE
                # work running behind them
                psv = ps_v.tile([128, 128], f32, tag="v")
                ttile(q, 0)
                for c in range(4):
                    for j in range(4):
                        nc.tensor.matmul(
                            psv[32 * j:32 * j + 16, :],
                            wvr[:, H * c:H * c + H],
                            xts(c, q, 128 * j, 128),
                            start=(c == 0), stop=False,
                            tile_position=(0, 32 * j),
                            skip_group_check=True)
                ttile(q, 1)
                for c in range(4, NDT):
                    for j in range(4):
                        nc.tensor.matmul(
                            psv[32 * j:32 * j + 16, :],
                            wvr[:, H * c:H * c + H],
                            xts(c, q, 128 * j, 128),
                            start=False, stop=(c == NDT - 1),
                            tile_position=(0, 32 * j),
                            skip_group_check=True)
                vt = vtsp.tile([128, 128], f16, tag="vts")
                svp = pers.tile([128, 1], f32, name=f"svp{q}", tag=f"svp{q}")
                nc.scalar.activation(vt[:], psv[:], Copy, accum_out=svp[:])
                vts[q] = vt
                svps.append(svp)
                ttile(q, 2)
                if q >= 1:
                    folds(q - 1)
                ttile(q, 3)
                if q >= 1:
                    xvmms(q - 1)
            folds(NQ - 1)
            psa = ps_v.tile([128, 128], f32, name="psa", tag="v")
            for _ in range(10):
                nc.tensor.matmul(psa[:, 0:16], dummy[:, 0:128],
                                 dummy[:, 0:16], start=True, stop=True,
                                 skip_group_check=True)
            xvmms(NQ - 1)

            # -------- A -> B transition (still inside A pools) --------
            xvt = pers.tile([128, 256], f16, tag="xvt")
            nc.vector.tensor_copy(xvt[:], psxv[:])

        # sv = sum_l v = strip/eighth partials + L*bv
        svt = pers.tile([128, 1], f32, tag="svt")
        nc.vector.tensor_add(svt[:], svps[0][:], svps[1][:])
        for q in range(2, NQ):
            nc.vector.tensor_add(svt[:], svt[:], svps[q][:])
        sv16 = pers.tile([H, 1], f32, tag="sv16")
        svf = []
        for j in range(1, 4):
            t = pers.tile([H, 1], f32, name=f"svf{j}", tag=f"svf{j}")
            nc.vector.tensor_copy(t[:], svt[32 * j:32 * j + 16, :])
            svf.append(t)
        nc.vector.tensor_add(sv16[:], svt[0:16, :], svf[0][:])
        nc.vector.tensor_add(sv16[:], sv16[:], svf[1][:])
        nc.vector.tensor_add(sv16[:], sv16[:], svf[2][:])
        bvl = pers.tile([H, 1], f32, tag="bvl")
        nc.scalar.mul(bvl[:], bvc[:], float(L))
        nc.vector.tensor_add(sv16[:], sv16[:], bvl[:])
        svst = pers.tile([128, 1], f32, tag="svst")
        for j in range(4):
            nc.vector.tensor_copy(svst[32 * j:32 * j + 16, :], sv16[:])
        bksv = pers.tile([128, 256], f32, tag="bksv")
        nc.scalar.activation(bksv[:], bkr[:], Copy, scale=svst[:])

        # ---------------- PHASE B ----------------
        with tc.tile_pool(name="sbB", bufs=1) as sbB, \
             tc.tile_pool(name="ps_3", bufs=1, space="PSUM") as ps_3, \
             tc.tile_pool(name="ps_4", bufs=1, space="PSUM") as ps_4, \
             tc.tile_pool(name="ps_m", bufs=2, space="PSUM") as ps_m, \
             tc.tile_pool(name="ps_c", bufs=1, space="PSUM") as ps_c, \
             tc.tile_pool(name="ps_z", bufs=1, space="PSUM") as ps_z:

            ktvt = sbB.tile([128, 256], f16, tag="ktvt")
            ut = sbB.tile([128, 256], f16, tag="ut")
            xvn = []
            ktvn = []
            un = []

            def warm(n):
                """dummy MMs to bridge PE waits on DVE/ACT evacs (keeps the
                stream dense and HAM at 8/8)."""
                psm = ps_m.tile([128, H], f32, tag="m")
                for _ in range(n):
                    nc.tensor.matmul(psm[:], dummy[:, 0:128], dummy[:, 0:16],
                                     start=True, stop=True,
                                     skip_group_check=True)

            warm(8)

            def tp16(src, d, out_name):
                """[16,128] strip-slice (chunk d) -> [128,16] via row-tiled
                normal matmul (out = slice.T @ I16)."""
                j = d // 2
                e = d % 2
                psm = ps_m.tile([128, H], f32, tag="m")
                nc.tensor.matmul(
                    psm[:], src[32 * j:32 * j + 16, 128 * e:128 * e + 128],
                    idq(j), start=True, stop=True,
                    skip_group_check=True, tile_position=(32 * j, 0))
                t = sbB.tile([128, H], f16, name=out_name, tag=out_name,
                             bufs=1)
                return psm, t

            # --- B pipeline: each stage's col-tiled groups run one item
            # behind the previous stage's strip-folds, so the PE stream
            # stays dense and the DVE copies hide underneath.
            ps3 = ps_3.tile([128, 256], f32, name="ps3", tag="s3")
            ps4 = ps_4.tile([128, 256], f32, name="ps4", tag="s4")
            psz0 = ps_z.tile([128, 512], f32, tag="z0")
            psz1 = ps_z.tile([128, 512], f32, tag="z1")
            psc = ps_c.tile([H, 1], f32, tag="c")
            sg0 = sbB.tile([128, 512], f16, tag="sg0")
            sg1 = sbB.tile([128, 512], f16, tag="sg1")

            def s3g(c):
                for jw in range(4):
                    nc.tensor.matmul(
                        ps3[32 * jw:32 * jw + 16, :], xvn[c][:],
                        wk_sb[:, 2048 * jw + 256 * c:2048 * jw + 256 * c + 256],
                        start=(c == 0), stop=(c == NDT - 1),
                        tile_position=(0, 32 * jw), skip_group_check=True)

            def s4g(c):
                for jw in range(4):
                    nc.tensor.matmul(
                        ps4[32 * jw:32 * jw + 16, :], ktvn[c][:],
                        wqt_sb[:, 2048 * jw + 256 * c:2048 * jw + 256 * c + 256],
                        start=(c == 0), stop=(c == NDT - 1),
                        tile_position=(0, 32 * jw), skip_group_check=True)
                nc.tensor.matmul(psc[:], ktvn[c][:], bqr[:, c:c + 1],
                                 start=(c == 0), stop=(c == NDT - 1))

            def zp(cc, ps, half):
                for j in range(4):
                    nc.tensor.matmul(
                        ps[32 * j:32 * j + 16, :], un[cc][:],
                        xts(cc, 2 * j + half), start=(cc == 0),
                        stop=(cc == NDT - 1),
                        tile_position=(0, 32 * j), skip_group_check=True)

            # stage 1: xv folds + step3 one-behind
            for c in range(NDT):
                psm, t = tp16(xvt, c, f"xvn{c}")
                nc.vector.tensor_copy(t[:], psm[:])
                xvn.append(t)
                if c >= 1:
                    s3g(c - 1)
            s3g(NDT - 1)
            nc.vector.tensor_add(ktvt[:], ps3[:], bksv[:])
            warm(6)

            # stage 2: ktv folds + step4/c one-behind
            for d in range(NDT):
                psm, t = tp16(ktvt, d, f"ktvn{d}")
                nc.vector.tensor_mul(t[:], psm[:], bdm[:, H * d:H * d + H])
                ktvn.append(t)
                if d >= 1:
                    s4g(d - 1)
            s4g(NDT - 1)
            nc.vector.tensor_copy(ut[:], ps4[:])
            c16 = sbB.tile([H, 1], f32, tag="c16")
            nc.scalar.copy(c16[:], psc[:])
            nc.scalar.mul(c16[:], c16[:], 0.125)
            cst = sbB.tile([128, 1], f32, tag="cst")
            for j in range(4):
                nc.vector.tensor_copy(cst[32 * j:32 * j + 16, :], c16[:])
            warm(6)

            # stage 3: u folds + z chunk groups one-behind
            for d in range(NDT):
                psm, t = tp16(ut, d, f"un{d}")
                nc.vector.tensor_copy(t[:], psm[:])
                un.append(t)
                if d >= 1:
                    zp(d - 1, psz0, 0)
                    zp(d - 1, psz1, 1)
            zp(NDT - 1, psz0, 0)
            nc.scalar.activation(sg0[:], psz0[:], Sigmoid, bias=cst[:],
                                 scale=0.125)
            nc.sync.dma_start(out0_d[:, :], sg0[:])
            zp(NDT - 1, psz1, 1)
            nc.scalar.activation(sg1[:], psz1[:], Sigmoid, bias=cst[:],
                                 scale=0.125)
            nc.scalar.dma_start(out1_d[:, :], sg1[:])
    return nc


B = 8
_cache = {}


def _get_nc():
    if "nc" not in _cache:
        _cache["nc"] = build()
    return _cache["nc"]


def build_in_maps(x, mask, Wq, bq, Wk, bk, Wv, bv):
    x16 = np.asarray(x).astype(np.float16)
    Wq = np.asarray(Wq, dtype=np.float32)
    Wk = np.asarray(Wk, dtype=np.float32)
    Wv = np.asarray(Wv, dtype=np.float32)
    bq = np.asarray(bq, dtype=np.float32)
    bk = np.asarray(bk, dtype=np.float32)
    bv = np.asarray(bv, dtype=np.float32)
    # wk/wqt d-major quarters: [p, (jw c n')]; row 128c+p, col 256jw+n'
    wk16 = np.ascontiguousarray(
        Wk.astype(np.float16).reshape(NDT, 128, 4, 256)
        .transpose(1, 2, 0, 3).reshape(128, 8192))
    wqt16 = np.ascontiguousarray(
        Wq.T.astype(np.float16).reshape(NDT, 128, 4, 256)
        .transpose(1, 2, 0, 3).reshape(128, 8192))
    wvr = np.ascontiguousarray(
        Wv.reshape(NDT, 128, H).transpose(1, 0, 2).reshape(128, NDT * H)
    ).astype(np.float16)
    bvr = np.ascontiguousarray(
        np.broadcast_to(bv[None, :], (128, H))).astype(np.float32)
    bqr = np.ascontiguousarray(bq.reshape(NDT, 128).T).astype(np.float16)
    bvc = np.ascontiguousarray(bv.reshape(H, 1))
    # bk strip layout [32j+h, n'] = bk[256j+n']
    bkr = np.ascontiguousarray(
        np.broadcast_to(bk.reshape(4, 1, 256), (4, 32, 256))
        .reshape(128, 256).astype(np.float32))
    # blockdiag masks, ktv-natural chunk-major [p, 16d+h]
    bdm = np.zeros((128, NDT * H), dtype=np.float32)
    for d in range(NDT):
        bdm[0:64, H * d + 2 * d] = 1.0
        bdm[64:128, H * d + 2 * d + 1] = 1.0
    in_maps = []
    for b in range(B):
        # xt: [p, (q c lq)] — x^T row 128c+p, col 512q+lq
        xtr = np.ascontiguousarray(
            x16[b].T.reshape(NDT, 128, NQ, 512)
            .transpose(1, 2, 0, 3).reshape(128, NQ * NDT * 512))
        in_maps.append({
            "xt": xtr, "wk": wk16, "wqt": wqt16, "wvr": wvr,
            "bvr": bvr, "bqr": bqr, "bvc": bvc, "bkr": bkr, "bdm": bdm,
        })
    return in_maps


def kernel(x, mask, Wq, bq, Wk, bk, Wv, bv):
    from concourse.bass_utils import run_bass_kernel_spmd
    nc = _get_nc()
    in_maps = build_in_maps(x, mask, Wq, bq, Wk, bk, Wv, bv)
    res = run_bass_kernel_spmd(nc, in_maps, core_ids=list(range(B)))
    outs = []
    for b in range(B):
        sg0 = np.asarray(res.results[b]["out0"], dtype=np.float32)
        sg1 = np.asarray(res.results[b]["out1"], dtype=np.float32)
        sg = np.concatenate([sg0, sg1], axis=1)      # [128, 1024]
        # row 32j+h, col n -> z[h, 1024j + n]
        zb = sg.reshape(4, 32, 1024)[:, 0:16].transpose(1, 0, 2).reshape(H, L)
        outs.append(zb)
    out = np.stack(outs, axis=0)
    out = out * np.asarray(mask).astype(np.float32)[:, None, :]
    return out.astype(np.float32)
